# revision 10
# baseline (speedup 1.0000x reference)
"""MultiHeadAttention Trainium2 kernel.

Full inputs -> full output. Sharding: 8 cores = (batch b in 0..3) x (head
group g in 0..1, 8 heads each). Each core projects Q/K/V for its head group
over all 2048 positions of batch b, runs attention for its 8 heads, applies
its half of the output projection (wo rows for its heads), and returns a
partial [2048, 1024] output. Host: full[b] = part(b,0) + part(b,1) + bias.

Everything stays in SBUF between phases (no DRAM round trips):
  phase A: project Q^T, K^T [512, 2048] (f32r, rank-1 bias matmuls) and the
           V table va [keys, head, 65] in bf16 with a ones column (so the
           softmax denominator falls out of the ctx matmul) scaled by
           exp(-1e9*mask) per key (exact mask semantics at zero cost).
  phase B: 32 software-pipelined (head, query-block) iterations:
           logits^T [sk, 512q] = K_h^T.T @ Q_h^T (f32r), P = exp(0.125*l)
           on the scalar engine -> bf16, ctx matmuls (bf16) accumulate
           [65, 512] (row 64 = denominator), DVE reciprocal, PE rank-1
           broadcast of the recips, DVE normalize-mul -> ctxn bf16 (odd
           heads take an identity matmul to shift to partitions 64..127).
  phase C: out_partial = ctxn @ wo (bf16 x bf16), f32 partials to DRAM.
"""

import numpy as np
import ml_dtypes

import concourse.bass as bass
import concourse.mybir as mybir
import concourse.tile as tile
from concourse import bacc
from concourse.bass_utils import run_bass_kernel_spmd

f32 = mybir.dt.float32
f32r = mybir.dt.float32r
bf16 = mybir.dt.bfloat16
np_bf16 = ml_dtypes.bfloat16

B, S, D, H, DH = 4, 2048, 1024, 16, 64
HG = H // 2          # 8 heads per core
DG = HG * DH         # 512 projection cols per core
N_CORES = 8
Exp = mybir.ActivationFunctionType.Exp

KC = D // 128        # 8 contraction chunks over d_model
CC = DG // 128       # 4 chunks over the head-group dim
SKC = S // 128       # 16 key chunks
NT = HG * 4          # 32 pipelined iterations (head, 512-query block)


def _build():
    nc = bacc.Bacc(None, target_bir_lowering=False)

    xq = nc.dram_tensor("xq", [D, S], f32r, kind="ExternalInput")   # query^T
    xk = nc.dram_tensor("xk", [D, S], f32r, kind="ExternalInput")   # key^T
    xv = nc.dram_tensor("xv", [D, S], f32r, kind="ExternalInput")   # value^T
    wq = nc.dram_tensor("wq", [D, DG], f32r, kind="ExternalInput")
    wk = nc.dram_tensor("wk", [D, DG], f32r, kind="ExternalInput")
    wv = nc.dram_tensor("wv", [D, DG], f32r, kind="ExternalInput")
    wo = nc.dram_tensor("wo", [DG, D], bf16, kind="ExternalInput")
    b2 = nc.dram_tensor("b2", [33, DG], f32r, kind="ExternalInput")  # bq row 0, bk row 32
    one = nc.dram_tensor("one", [33, 512], f32r, kind="ExternalInput")
    emask8 = nc.dram_tensor("emask8", [128, SKC, HG], bf16, kind="ExternalInput")
    emaskf = nc.dram_tensor("emaskf", [128, SKC], f32, kind="ExternalInput")
    ident = nc.dram_tensor("ident", [64, 128], bf16, kind="ExternalInput")
    oneb = nc.dram_tensor("oneb", [1, 64], bf16, kind="ExternalInput")
    out = nc.dram_tensor("out", [S, D], f32, kind="ExternalOutput")

    with tile.TileContext(nc) as tc:
        _emit(nc, tc, xq, xk, xv, wq, wk, wv, wo, b2, one, emask8, emaskf,
              ident, oneb, out)
    nc.finalize()
    return nc


def _emit(nc, tc, xq, xk, xv, wq, wk, wv, wo, b2, one, emask8, emaskf,
          ident, oneb, out):
    from contextlib import ExitStack

    with ExitStack() as ctx:
        consts = ctx.enter_context(tc.tile_pool(name="consts", bufs=1))
        wpool = ctx.enter_context(tc.tile_pool(name="wpool", bufs=2))
        xtp = ctx.enter_context(tc.tile_pool(name="xtp", bufs=2))
        big = ctx.enter_context(tc.tile_pool(name="big", bufs=1))
        ptp = ctx.enter_context(tc.tile_pool(name="ptp", bufs=12))
        tmp = ctx.enter_context(tc.tile_pool(name="tmp", bufs=1))
        rcp = ctx.enter_context(tc.tile_pool(name="rcp", bufs=1))
        stg = ctx.enter_context(tc.tile_pool(name="stg", bufs=2))
        psl = ctx.enter_context(tc.tile_pool(name="psl", bufs=2, space="PSUM"))
        psc = ctx.enter_context(tc.tile_pool(name="psc", bufs=2, space="PSUM"))
        psx = ctx.enter_context(tc.tile_pool(name="psx", bufs=1, space="PSUM"))
        psq = ctx.enter_context(tc.tile_pool(name="psq", bufs=1, space="PSUM"))

        b2_sb = consts.tile([33, DG], f32r)
        nc.sync.dma_start(b2_sb, b2[:])
        ones = consts.tile([33, 512], f32r)
        nc.sync.dma_start(ones, one[:])
        em_sb = consts.tile([128, SKC], f32)
        nc.sync.dma_start(em_sb, emaskf[:])
        id_sb = consts.tile([64, 128], bf16)
        nc.sync.dma_start(id_sb, ident[:])
        onesb = consts.tile([1, 64], bf16)
        nc.sync.dma_start(onesb, oneb[:])

        qt_sb = big.tile([128, CC, S], bf16)        # Q^T: head h at [(h%2)*64, h//2]
        kt_sb = big.tile([128, CC, S], bf16)        # K^T: same layout
        va_sb = big.tile([128, SKC, HG, DH + 1], bf16)  # [v*em, em] per key/head
        cx_sb = big.tile([128, CC, S], bf16)        # normalized ctx^T

        # ones column of va = exp(-1e9*mask) per key
        nc.sync.dma_start(va_sb[:, :, :, DH], emask8[:])

        # ================= phase A: projections =================
        wqt = wpool.tile([128, KC, DG], f32r, tag="w", name="wqt")
        nc.sync.dma_start(wqt, wq[:].rearrange("(ko p) c -> p ko c", p=128))
        wkt = wpool.tile([128, KC, DG], f32r, tag="w", name="wkt")
        nc.sync.dma_start(wkt, wk[:].rearrange("(ko p) c -> p ko c", p=128))

        def load_xblk(x_dram, blk):
            xT = xtp.tile([128, KC, 512], f32r, tag="xT", name="xT")
            nc.sync.dma_start(xT, x_dram[:, blk * 512:(blk + 1) * 512]
                              .rearrange("(ko p) s -> p ko s", p=128))
            return xT

        def proj_block(wt, brow, xT, dst_sb, cc, blk, ps=None, part=2):
            """One [128, 512] block of Q^T/K^T; part 0/1 = split halves."""
            if part != 1:
                ps = psq.tile([128, 512], f32, tag="psq", name="ps")
            for kc in (range(0, 4) if part == 0 else
                       range(4, KC) if part == 1 else range(KC)):
                nc.tensor.matmul(ps, lhsT=wt[:, kc, cc * 128:(cc + 1) * 128],
                                 rhs=xT[:, kc, :],
                                 start=(kc == 0), stop=False)
            if part == 0:
                return ps
            nc.tensor.matmul(ps, lhsT=b2_sb[brow:brow + 1, cc * 128:(cc + 1) * 128],
                             rhs=ones[brow:brow + 1, 0:512],
                             start=False, stop=True)
            with nc.allow_low_precision(reason="proj rounded to bf16"):
                nc.vector.tensor_copy(
                    dst_sb[:, cc, blk * 512:(blk + 1) * 512], ps)
            return None

        for blk in range(4):
            xT = load_xblk(xk, blk)
            for cc in range(CC):
                proj_block(wkt, 32, xT, kt_sb, cc, blk)
        xT0 = load_xblk(xq, 0)
        for cc in range(CC):
            proj_block(wqt, 0, xT0, qt_sb, cc, 0)
        q_tiles = {1: load_xblk(xq, 1)}

        # V: [keys, 8h*64] scaled by emask per key, interleaved into va
        wvt = wpool.tile([128, KC, DG], f32r, tag="w", name="wvt")
        nc.sync.dma_start(wvt, wv[:].rearrange("(ko p) c -> p ko c", p=128))
        for sc in range(SKC):
            xvt = xtp.tile([128, KC, 128], f32r, tag="xT", name="xvt")
            nc.sync.dma_start(xvt, xv[:, sc * 128:(sc + 1) * 128]
                              .rearrange("(ko p) s -> p ko s", p=128))
            ps = psx.tile([128, 512], f32, tag="psx", name="ps")
            for kc in range(KC):
                nc.tensor.matmul(ps, lhsT=xvt[:, kc, :], rhs=wvt[:, kc, :],
                                 start=(kc == 0), stop=(kc == KC - 1))
            with nc.allow_low_precision(reason="va in bf16"):
                nc.vector.tensor_scalar_mul(
                    va_sb[:, sc, :, 0:DH],
                    ps.rearrange("p (h d) -> p h d", h=HG),
                    em_sb[:, sc:sc + 1])

        # ================= phase B: pipelined attention =================
        state = {}

        def emit_logits_pair(t, kcp):
            st_ = state[t]
            h, sqb = st_["h"], st_["sqb"]
            hp, hcc = (h % 2) * 64, h // 2
            ps_ = psl.tile([128, 1024], f32, tag="psl", name="psl")
            for half in range(2):
                skc = kcp * 2 + half
                nc.tensor.matmul(ps_[:, half * 512:(half + 1) * 512],
                                 lhsT=kt_sb[hp:hp + 64, hcc,
                                            skc * 128:(skc + 1) * 128],
                                 rhs=qt_sb[hp:hp + 64, hcc,
                                           sqb * 512:(sqb + 1) * 512],
                                 start=True, stop=True)
            pt = ptp.tile([128, 2, 512], bf16, tag="pt", name="pt")
            nc.scalar.activation(pt.rearrange("p a b -> p (a b)"), ps_, Exp,
                                 scale=0.125)
            st_["pt"].append(pt)

        def emit_ctx_chunk(t, skc):
            st_ = state[t]
            if skc == 0:
                st_["psc"] = psc.tile([128, 512], f32, tag="psc", name="psc")
            nc.tensor.matmul(st_["psc"][0:DH + 1, :],
                             lhsT=va_sb[:, skc, st_["h"], :],
                             rhs=st_["pt"][skc // 2][:, skc % 2, :],
                             start=(skc == 0), stop=(skc == SKC - 1))

        def emit_norm(t):
            st_ = state[t]
            h, sqb = st_["h"], st_["sqb"]
            hcc, odd = h // 2, h % 2
            cu = stg.tile([DH + 1, 512], f32, tag="cu", name="cu")
            nc.vector.tensor_copy(cu, st_["psc"][0:DH + 1, :])
            rec = rcp.tile([1, 512], bf16, tag="rec", name="rec")
            with nc.allow_low_precision(reason="recip rounded to f32r"):
                nc.vector.reciprocal(rec, cu[DH:DH + 1, :])
            bc = psx.tile([128, 512], f32, tag="psx", name="bc")
            nc.tensor.matmul(bc[0:64, :], lhsT=onesb[0:1, :], rhs=rec[:],
                             start=True, stop=True)
            dst = cx_sb[64 * odd:64 * odd + 64, hcc, sqb * 512:(sqb + 1) * 512]
            with nc.allow_low_precision(reason="ctxn in bf16"):
                if not odd:
                    nc.vector.tensor_mul(out=dst, in0=cu[0:DH, :],
                                         in1=bc[0:64, :])
                else:
                    tm = tmp.tile([64, 512], bf16, tag="tmp", name="tm")
                    nc.vector.tensor_mul(out=tm, in0=cu[0:DH, :],
                                         in1=bc[0:64, :])
                    sh = psx.tile([128, 512], f32, tag="psx", name="sh")
                    nc.tensor.matmul(sh, lhsT=id_sb[:], rhs=tm[:],
                                     start=True, stop=True)
                    nc.vector.tensor_copy(dst, sh[64:128, :])
            del state[t]

        qp_ps = None
        for t in range(NT):
            sqb, h = divmod(t, 8)
            state[t] = {"h": h, "sqb": sqb, "pt": []}
            if h == 6 and sqb < 2:
                q_tiles[sqb + 2] = load_xblk(xq, sqb + 2)
            for kcp in range(SKC // 2):
                emit_logits_pair(t, kcp)
                if t >= 1:
                    emit_ctx_chunk(t - 1, kcp * 2)
                    emit_ctx_chunk(t - 1, kcp * 2 + 1)
                if sqb < 3 and kcp == 3:
                    if h % 2 == 0:
                        qp_ps = proj_block(wqt, 0, q_tiles[sqb + 1], qt_sb,
                                           h // 2, sqb + 1, part=0)
                    else:
                        proj_block(wqt, 0, q_tiles[sqb + 1], qt_sb,
                                   h // 2, sqb + 1, ps=qp_ps, part=1)
            if t >= 1:
                emit_norm(t - 1)
        for skc in range(SKC):
            emit_ctx_chunk(NT - 1, skc)
        emit_norm(NT - 1)

        # ================= phase C: output projection =================
        wot = wpool.tile([128, CC, D], bf16, tag="w", name="wot")
        nc.sync.dma_start(wot, wo[:].rearrange("(co p) c -> p co c", p=128))
        for st8 in range(SKC):
            ot = stg.tile([128, 1024], f32, tag="ost", name="ot")
            for half in range(2):
                ps = psx.tile([128, 512], f32, tag="psx", name="ps")
                for cc in range(CC):
                    nc.tensor.matmul(ps,
                                     lhsT=cx_sb[:, cc, st8 * 128:(st8 + 1) * 128],
                                     rhs=wot[:, cc, half * 512:(half + 1) * 512],
                                     start=(cc == 0), stop=(cc == CC - 1))
                nc.vector.tensor_copy(ot[:, half * 512:(half + 1) * 512], ps)
            nc.sync.dma_start(out[st8 * 128:(st8 + 1) * 128, :], ot)


_NC_CACHE = None


def kernel(query, key, value, mask, wq, bq, wk, bk, wv, bv, wo, bo):
    global _NC_CACHE
    if _NC_CACHE is None:
        _NC_CACHE = _build()
    nc = _NC_CACHE

    query = np.asarray(query, dtype=np.float32)
    key = np.asarray(key, dtype=np.float32)
    value = np.asarray(value, dtype=np.float32)
    mask = np.asarray(mask, dtype=np.float32)
    wq_np = np.asarray(wq, np.float32)
    wk_np = np.asarray(wk, np.float32)
    wv_np = np.asarray(wv, np.float32)
    wo_np = np.asarray(wo, np.float32)
    bq_np = np.asarray(bq, np.float32)
    bk_np = np.asarray(bk, np.float32)
    # fold bv and bo through the output projection (added on host at the end)
    bias_out = (np.asarray(bo, np.float64) +
                np.asarray(bv, np.float64) @ np.asarray(wo_np, np.float64)
                ).astype(np.float32)

    xT = {}
    for b in range(B):
        xT[b] = (np.ascontiguousarray(query[b].T),
                 np.ascontiguousarray(key[b].T),
                 np.ascontiguousarray(value[b].T))
    shared_g = []
    for g in range(2):
        cols = slice(DG * g, DG * (g + 1))
        b2_host = np.zeros((33, DG), np.float32)
        b2_host[0] = bq_np[cols]
        b2_host[32] = bk_np[cols]
        shared_g.append({
            "wq": np.ascontiguousarray(wq_np[:, cols]),
            "wk": np.ascontiguousarray(wk_np[:, cols]),
            "wv": np.ascontiguousarray(wv_np[:, cols]),
            "wo": np.ascontiguousarray(wo_np[cols, :]).astype(np_bf16),
            "b2": np.ascontiguousarray(b2_host),
        })
    one_host = np.ones((33, 512), np.float32)
    id_host = np.concatenate([np.zeros((64, 64), np.float32),
                              np.eye(64, dtype=np.float32)], axis=1).astype(np_bf16)
    oneb_host = np.ones((1, 64), np_bf16)

    in_maps = []
    for core in range(N_CORES):
        b, g = divmod(core, 2)
        em = np.exp(mask[b, 0, 0] * np.float32(-1e9)).astype(np.float32)
        emc = np.ascontiguousarray(em.reshape(SKC, 128).T)   # [128, SKC]
        em8 = np.ascontiguousarray(
            np.repeat(emc[:, :, None], HG, axis=2)).astype(np_bf16)
        in_maps.append({
            "xq": xT[b][0], "xk": xT[b][1], "xv": xT[b][2],
            "emask8": em8, "emaskf": emc,
            "one": one_host, "ident": id_host, "oneb": oneb_host,
            **shared_g[g],
        })

    res = run_bass_kernel_spmd(nc, in_maps, core_ids=list(range(N_CORES)))
    full = np.empty((B, S, D), np.float32)
    for b in range(B):
        full[b] = res.results[2 * b]["out"]
        full[b] += res.results[2 * b + 1]["out"]
        full[b] += bias_out
    return full


# revision 12
# speedup vs baseline: 1.2867x; 1.2867x over previous
"""MultiHeadAttention Trainium2 kernel.

Full inputs -> full output. Sharding: 8 cores = (batch b in 0..3) x (head
group g in 0..1, 8 heads each). Each core projects Q/K/V for its head group
over all 2048 positions of batch b, runs attention for its 8 heads, applies
its half of the output projection (wo rows for its heads), and returns a
partial [2048, 1024] output. Host: full[b] = part(b,0) + part(b,1) + bias.

Uniform bf16 datapath (fp32 PSUM accumulation), everything SBUF-resident
between phases (no DRAM round trips) to stay under the chip power envelope:
  phase A: project Q^T, K^T [512, 2048] (rank-1 bias matmuls) and the
           V table va [keys, head, 65] with a ones column (so the softmax
           denominator falls out of the ctx matmul) scaled by
           exp(-1e9*mask) per key (exact mask semantics at zero cost).
  phase B: 32 software-pipelined (head, query-block) iterations:
           logits^T [sk, 512q] = K_h^T.T @ Q_h^T, P = exp(0.125*l) on the
           scalar engine -> bf16, ctx matmuls accumulate [65, 512] (row 64
           = denominator), DVE fast-reciprocal, PE rank-1 broadcast of the
           recips, DVE normalize-mul -> ctxn bf16 (odd heads go through a
           shifted-identity matmul to land at partitions 64..127).
  phase C: out_partial = ctxn @ wo, f32 partials to DRAM.
"""

import numpy as np
import ml_dtypes

import concourse.bass as bass
import concourse.mybir as mybir
import concourse.tile as tile
from concourse import bacc
from concourse.bass_utils import run_bass_kernel_spmd

f32 = mybir.dt.float32
bf16 = mybir.dt.bfloat16
np_bf16 = ml_dtypes.bfloat16

B, S, D, H, DH = 4, 2048, 1024, 16, 64
HG = H // 2          # 8 heads per core
DG = HG * DH         # 512 projection cols per core
N_CORES = 8
Exp = mybir.ActivationFunctionType.Exp

KC = D // 128        # 8 contraction chunks over d_model
CC = DG // 128       # 4 chunks over the head-group dim
SKC = S // 128       # 16 key chunks
NT = HG * 4          # 32 pipelined iterations (head, 512-query block)


def _build():
    nc = bacc.Bacc(None, target_bir_lowering=False)

    xq = nc.dram_tensor("xq", [D, S], bf16, kind="ExternalInput")   # query^T
    xk = nc.dram_tensor("xk", [D, S], bf16, kind="ExternalInput")   # key^T
    xv = nc.dram_tensor("xv", [D, S], bf16, kind="ExternalInput")   # value^T
    wq = nc.dram_tensor("wq", [D, DG], bf16, kind="ExternalInput")
    wk = nc.dram_tensor("wk", [D, DG], bf16, kind="ExternalInput")
    wv = nc.dram_tensor("wv", [D, DG], bf16, kind="ExternalInput")
    wo = nc.dram_tensor("wo", [DG, D], bf16, kind="ExternalInput")
    b2 = nc.dram_tensor("b2", [33, DG], bf16, kind="ExternalInput")  # bq@0, bk@32
    one = nc.dram_tensor("one", [33, 512], bf16, kind="ExternalInput")
    emask8 = nc.dram_tensor("emask8", [128, SKC, HG], bf16, kind="ExternalInput")
    emaskf = nc.dram_tensor("emaskf", [128, SKC], f32, kind="ExternalInput")
    ident = nc.dram_tensor("ident", [64, 128], bf16, kind="ExternalInput")
    out = nc.dram_tensor("out", [S, D], f32, kind="ExternalOutput")

    with tile.TileContext(nc) as tc:
        _emit(nc, tc, xq, xk, xv, wq, wk, wv, wo, b2, one, emask8, emaskf,
              ident, out)
    nc.finalize()
    return nc


def _emit(nc, tc, xq, xk, xv, wq, wk, wv, wo, b2, one, emask8, emaskf,
          ident, out):
    from contextlib import ExitStack

    with ExitStack() as ctx:
        consts = ctx.enter_context(tc.tile_pool(name="consts", bufs=1))
        wpool = ctx.enter_context(tc.tile_pool(name="wpool", bufs=2))
        xtp = ctx.enter_context(tc.tile_pool(name="xtp", bufs=2))
        big = ctx.enter_context(tc.tile_pool(name="big", bufs=1))
        ptp = ctx.enter_context(tc.tile_pool(name="ptp", bufs=16))
        tmp = ctx.enter_context(tc.tile_pool(name="tmp", bufs=1))
        rcp = ctx.enter_context(tc.tile_pool(name="rcp", bufs=2))
        stg = ctx.enter_context(tc.tile_pool(name="stg", bufs=2))
        psl = ctx.enter_context(tc.tile_pool(name="psl", bufs=2, space="PSUM"))
        psc = ctx.enter_context(tc.tile_pool(name="psc", bufs=2, space="PSUM"))
        psx = ctx.enter_context(tc.tile_pool(name="psx", bufs=2, space="PSUM"))

        b2_sb = consts.tile([33, DG], bf16)
        nc.sync.dma_start(b2_sb, b2[:])
        ones = consts.tile([33, 512], bf16)
        nc.sync.dma_start(ones, one[:])
        em_sb = consts.tile([128, SKC], f32)
        nc.sync.dma_start(em_sb, emaskf[:])
        id_sb = consts.tile([64, 128], bf16)
        nc.sync.dma_start(id_sb, ident[:])

        qt_sb = big.tile([128, CC, S], bf16)        # Q^T: head h at [(h%2)*64, h//2]
        kt_sb = big.tile([128, CC, S], bf16)        # K^T: same layout
        va_sb = big.tile([128, SKC, HG, DH + 1], bf16)  # [v*em, em] per key/head
        cx_sb = big.tile([128, CC, S], bf16)        # normalized ctx^T

        # ones column of va = exp(-1e9*mask) per key
        nc.sync.dma_start(va_sb[:, :, :, DH], emask8[:])

        # ================= phase A: projections =================
        def project_T(w_dram, brow, x_dram, dst_sb):
            """Q^T / K^T [512, 2048] = w_g^T @ x^T, bias via rank-1 matmul."""
            wt = wpool.tile([128, KC, DG], bf16, tag="w", name="wt")
            nc.sync.dma_start(wt, w_dram[:].rearrange("(ko p) c -> p ko c", p=128))
            for blk in range(4):
                xT = xtp.tile([128, KC, 512], bf16, tag="xT", name="xT")
                nc.sync.dma_start(xT, x_dram[:, blk * 512:(blk + 1) * 512]
                                  .rearrange("(ko p) s -> p ko s", p=128))
                for cc in range(CC):
                    ps = psx.tile([128, 512], f32, tag="psx", name="ps")
                    for kc in range(KC):
                        nc.tensor.matmul(ps, lhsT=wt[:, kc, cc * 128:(cc + 1) * 128],
                                         rhs=xT[:, kc, :],
                                         start=(kc == 0), stop=False)
                    nc.tensor.matmul(ps, lhsT=b2_sb[brow:brow + 1, cc * 128:(cc + 1) * 128],
                                     rhs=ones[brow:brow + 1, 0:512],
                                     start=False, stop=True)
                    with nc.allow_low_precision(reason="proj rounded to bf16"):
                        nc.vector.tensor_copy(
                            dst_sb[:, cc, blk * 512:(blk + 1) * 512], ps)

        project_T(wq, 0, xq, qt_sb)
        project_T(wk, 32, xk, kt_sb)

        # V: [keys, 8h*64] scaled by emask per key, interleaved into va
        wvt = wpool.tile([128, KC, DG], bf16, tag="w", name="wvt")
        nc.sync.dma_start(wvt, wv[:].rearrange("(ko p) c -> p ko c", p=128))
        for sc in range(SKC):
            xvt = xtp.tile([128, KC, 128], bf16, tag="xT", name="xvt")
            nc.sync.dma_start(xvt, xv[:, sc * 128:(sc + 1) * 128]
                              .rearrange("(ko p) s -> p ko s", p=128))
            ps = psx.tile([128, 512], f32, tag="psx", name="ps")
            for kc in range(KC):
                nc.tensor.matmul(ps, lhsT=xvt[:, kc, :], rhs=wvt[:, kc, :],
                                 start=(kc == 0), stop=(kc == KC - 1))
            with nc.allow_low_precision(reason="va in bf16"):
                nc.vector.tensor_scalar_mul(
                    va_sb[:, sc, :, 0:DH],
                    ps.rearrange("p (h d) -> p h d", h=HG),
                    em_sb[:, sc:sc + 1])

        # ================= phase B: pipelined attention =================
        state = {}

        def emit_logits_pair(t, kcp):
            st_ = state[t]
            h, sqb = st_["h"], st_["sqb"]
            hp, hcc = (h % 2) * 64, h // 2
            ps_ = psl.tile([128, 1024], f32, tag="psl", name="psl")
            for half in range(2):
                skc = kcp * 2 + half
                nc.tensor.matmul(ps_[:, half * 512:(half + 1) * 512],
                                 lhsT=kt_sb[hp:hp + 64, hcc,
                                            skc * 128:(skc + 1) * 128],
                                 rhs=qt_sb[hp:hp + 64, hcc,
                                           sqb * 512:(sqb + 1) * 512],
                                 start=True, stop=True)
            pt = ptp.tile([128, 2, 512], bf16, tag="pt", name="pt")
            nc.scalar.activation(pt.rearrange("p a b -> p (a b)"), ps_, Exp,
                                 scale=0.125)
            st_["pt"].append(pt)

        def emit_ctx_chunk(t, skc):
            st_ = state[t]
            if skc == 0:
                st_["psc"] = psc.tile([128, 512], f32, tag="psc", name="psc")
            nc.tensor.matmul(st_["psc"][0:DH + 1, :],
                             lhsT=va_sb[:, skc, st_["h"], :],
                             rhs=st_["pt"][skc // 2][:, skc % 2, :],
                             start=(skc == 0), stop=(skc == SKC - 1))

        def emit_norm(t):
            st_ = state[t]
            h, sqb = st_["h"], st_["sqb"]
            hcc, odd = h // 2, h % 2
            cu = stg.tile([DH + 1, 512], f32, tag="cu", name="cu")
            nc.vector.tensor_copy(cu, st_["psc"][0:DH + 1, :])
            rec = rcp.tile([1, 512], bf16, tag="rec", name="rec")
            with nc.allow_low_precision(reason="recip rounded to bf16"):
                nc.vector.reciprocal(rec, cu[DH:DH + 1, :])
            bc = psx.tile([128, 512], f32, tag="psx", name="bc")
            nc.tensor.matmul(bc[0:64, :], lhsT=ones[0:1, 0:64], rhs=rec[:],
                             start=True, stop=True)
            dst = cx_sb[64 * odd:64 * odd + 64, hcc, sqb * 512:(sqb + 1) * 512]
            with nc.allow_low_precision(reason="ctxn in bf16"):
                if not odd:
                    nc.vector.tensor_mul(out=dst, in0=cu[0:DH, :],
                                         in1=bc[0:64, :])
                else:
                    tm = tmp.tile([64, 512], bf16, tag="tmp", name="tm")
                    nc.vector.tensor_mul(out=tm, in0=cu[0:DH, :],
                                         in1=bc[0:64, :])
                    sh = psx.tile([128, 512], f32, tag="psx", name="sh")
                    nc.tensor.matmul(sh, lhsT=id_sb[:], rhs=tm[:],
                                     start=True, stop=True)
                    nc.vector.tensor_copy(dst, sh[64:128, :])
            del state[t]

        for t in range(NT):
            h, sqb = divmod(t, 4)
            state[t] = {"h": h, "sqb": sqb, "pt": []}
            for kcp in range(SKC // 2):
                emit_logits_pair(t, kcp)
                if t >= 1:
                    emit_ctx_chunk(t - 1, kcp * 2)
                    emit_ctx_chunk(t - 1, kcp * 2 + 1)
            if t >= 1:
                emit_norm(t - 1)
        for skc in range(SKC):
            emit_ctx_chunk(NT - 1, skc)
        emit_norm(NT - 1)

        # ================= phase C: output projection =================
        wot = wpool.tile([128, CC, D], bf16, tag="w", name="wot")
        nc.sync.dma_start(wot, wo[:].rearrange("(co p) c -> p co c", p=128))
        for st8 in range(SKC):
            ot = stg.tile([128, 1024], f32, tag="ost", name="ot")
            for half in range(2):
                ps = psx.tile([128, 512], f32, tag="psx", name="ps")
                for cc in range(CC):
                    nc.tensor.matmul(ps,
                                     lhsT=cx_sb[:, cc, st8 * 128:(st8 + 1) * 128],
                                     rhs=wot[:, cc, half * 512:(half + 1) * 512],
                                     start=(cc == 0), stop=(cc == CC - 1))
                nc.vector.tensor_copy(ot[:, half * 512:(half + 1) * 512], ps)
            nc.sync.dma_start(out[st8 * 128:(st8 + 1) * 128, :], ot)


_NC_CACHE = None


def kernel(query, key, value, mask, wq, bq, wk, bk, wv, bv, wo, bo):
    global _NC_CACHE
    if _NC_CACHE is None:
        _NC_CACHE = _build()
    nc = _NC_CACHE

    query = np.asarray(query, dtype=np.float32)
    key = np.asarray(key, dtype=np.float32)
    value = np.asarray(value, dtype=np.float32)
    mask = np.asarray(mask, dtype=np.float32)
    wq_np = np.asarray(wq, np.float32)
    wk_np = np.asarray(wk, np.float32)
    wv_np = np.asarray(wv, np.float32)
    wo_np = np.asarray(wo, np.float32)
    bq_np = np.asarray(bq, np.float32)
    bk_np = np.asarray(bk, np.float32)
    # fold bv and bo through the output projection (added on host at the end)
    bias_out = (np.asarray(bo, np.float64) +
                np.asarray(bv, np.float64) @ np.asarray(wo_np, np.float64)
                ).astype(np.float32)

    xT = {}
    for b in range(B):
        xT[b] = (np.ascontiguousarray(query[b].T).astype(np_bf16),
                 np.ascontiguousarray(key[b].T).astype(np_bf16),
                 np.ascontiguousarray(value[b].T).astype(np_bf16))
    shared_g = []
    for g in range(2):
        cols = slice(DG * g, DG * (g + 1))
        b2_host = np.zeros((33, DG), np.float32)
        b2_host[0] = bq_np[cols]
        b2_host[32] = bk_np[cols]
        shared_g.append({
            "wq": np.ascontiguousarray(wq_np[:, cols]).astype(np_bf16),
            "wk": np.ascontiguousarray(wk_np[:, cols]).astype(np_bf16),
            "wv": np.ascontiguousarray(wv_np[:, cols]).astype(np_bf16),
            "wo": np.ascontiguousarray(wo_np[cols, :]).astype(np_bf16),
            "b2": b2_host.astype(np_bf16),
        })
    one_host = np.ones((33, 512), np_bf16)
    id_host = np.concatenate([np.zeros((64, 64), np.float32),
                              np.eye(64, dtype=np.float32)],
                             axis=1).astype(np_bf16)

    in_maps = []
    for core in range(N_CORES):
        b, g = divmod(core, 2)
        em = np.exp(mask[b, 0, 0] * np.float32(-1e9)).astype(np.float32)
        emc = np.ascontiguousarray(em.reshape(SKC, 128).T)   # [128, SKC]
        em8 = np.ascontiguousarray(
            np.repeat(emc[:, :, None], HG, axis=2)).astype(np_bf16)
        in_maps.append({
            "xq": xT[b][0], "xk": xT[b][1], "xv": xT[b][2],
            "emask8": em8, "emaskf": emc,
            "one": one_host, "ident": id_host,
            **shared_g[g],
        })

    res = run_bass_kernel_spmd(nc, in_maps, core_ids=list(range(N_CORES)))
    full = np.empty((B, S, D), np.float32)
    for b in range(B):
        full[b] = res.results[2 * b]["out"]
        full[b] += res.results[2 * b + 1]["out"]
        full[b] += bias_out
    return full


# revision 13
# speedup vs baseline: 1.3019x; 1.0118x over previous
"""MultiHeadAttention Trainium2 kernel.

Full inputs -> full output. Sharding: 8 cores = (batch b in 0..3) x (head
group g in 0..1, 8 heads each). Each core projects Q/K/V for its head group
over all 2048 positions of batch b, runs attention for its 8 heads, applies
its half of the output projection (wo rows for its heads), and returns a
partial [2048, 1024] output. Host: full[b] = part(b,0) + part(b,1) + bias.

Uniform bf16 datapath (fp32 PSUM accumulation), everything SBUF-resident
between phases (no DRAM round trips) to stay under the chip power envelope:
  phase A: project Q^T, K^T [512, 2048] (rank-1 bias matmuls) and the
           V table va [keys, head, 65] with a ones column (so the softmax
           denominator falls out of the ctx matmul) scaled by
           exp(-1e9*mask) per key (exact mask semantics at zero cost).
  phase B: 32 software-pipelined (head, query-block) iterations:
           logits^T [sk, 512q] = K_h^T.T @ Q_h^T, P = exp(0.125*l) on the
           scalar engine -> bf16, ctx matmuls accumulate [65, 512] (row 64
           = denominator), DVE fast-reciprocal, PE rank-1 broadcast of the
           recips, DVE normalize-mul -> ctxn bf16 (odd heads go through a
           shifted-identity matmul to land at partitions 64..127).
  phase C: out_partial = ctxn @ wo, f32 partials to DRAM.
"""

import numpy as np
import ml_dtypes

import concourse.bass as bass
import concourse.mybir as mybir
import concourse.tile as tile
from concourse import bacc
from concourse.bass_utils import run_bass_kernel_spmd

f32 = mybir.dt.float32
bf16 = mybir.dt.bfloat16
np_bf16 = ml_dtypes.bfloat16

B, S, D, H, DH = 4, 2048, 1024, 16, 64
HG = H // 2          # 8 heads per core
DG = HG * DH         # 512 projection cols per core
N_CORES = 8
Exp = mybir.ActivationFunctionType.Exp

KC = D // 128        # 8 contraction chunks over d_model
CC = DG // 128       # 4 chunks over the head-group dim
SKC = S // 128       # 16 key chunks
NT = HG * 4          # 32 pipelined iterations (head, 512-query block)


def _build():
    nc = bacc.Bacc(None, target_bir_lowering=False)

    xq = nc.dram_tensor("xq", [D, S], bf16, kind="ExternalInput")   # query^T
    xk = nc.dram_tensor("xk", [D, S], bf16, kind="ExternalInput")   # key^T
    xv = nc.dram_tensor("xv", [D, S], bf16, kind="ExternalInput")   # value^T
    wq = nc.dram_tensor("wq", [D, DG], bf16, kind="ExternalInput")
    wk = nc.dram_tensor("wk", [D, DG], bf16, kind="ExternalInput")
    wv = nc.dram_tensor("wv", [D, DG], bf16, kind="ExternalInput")
    wo = nc.dram_tensor("wo", [DG, D], bf16, kind="ExternalInput")
    b2 = nc.dram_tensor("b2", [33, DG], bf16, kind="ExternalInput")  # bq@0, bk@32
    one = nc.dram_tensor("one", [33, 512], bf16, kind="ExternalInput")
    emask8 = nc.dram_tensor("emask8", [128, SKC, HG], bf16, kind="ExternalInput")
    emaskf = nc.dram_tensor("emaskf", [128, SKC], f32, kind="ExternalInput")
    ident = nc.dram_tensor("ident", [64, 128], bf16, kind="ExternalInput")
    out = nc.dram_tensor("out", [S, D], f32, kind="ExternalOutput")

    with tile.TileContext(nc) as tc:
        _emit(nc, tc, xq, xk, xv, wq, wk, wv, wo, b2, one, emask8, emaskf,
              ident, out)
    nc.finalize()
    return nc


def _emit(nc, tc, xq, xk, xv, wq, wk, wv, wo, b2, one, emask8, emaskf,
          ident, out):
    from contextlib import ExitStack

    with ExitStack() as ctx:
        consts = ctx.enter_context(tc.tile_pool(name="consts", bufs=1))
        wpool = ctx.enter_context(tc.tile_pool(name="wpool", bufs=2))
        xtp = ctx.enter_context(tc.tile_pool(name="xtp", bufs=2))
        big = ctx.enter_context(tc.tile_pool(name="big", bufs=1))
        ptp = ctx.enter_context(tc.tile_pool(name="ptp", bufs=16))
        tmp = ctx.enter_context(tc.tile_pool(name="tmp", bufs=1))
        rcp = ctx.enter_context(tc.tile_pool(name="rcp", bufs=2))
        stg = ctx.enter_context(tc.tile_pool(name="stg", bufs=2))
        psl = ctx.enter_context(tc.tile_pool(name="psl", bufs=2, space="PSUM"))
        psc = ctx.enter_context(tc.tile_pool(name="psc", bufs=3, space="PSUM"))
        psx = ctx.enter_context(tc.tile_pool(name="psx", bufs=1, space="PSUM"))

        b2_sb = consts.tile([33, DG], bf16)
        nc.sync.dma_start(b2_sb, b2[:])
        ones = consts.tile([33, 512], bf16)
        nc.sync.dma_start(ones, one[:])
        em_sb = consts.tile([128, SKC], f32)
        nc.sync.dma_start(em_sb, emaskf[:])
        id_sb = consts.tile([64, 128], bf16)
        nc.sync.dma_start(id_sb, ident[:])

        qt_sb = big.tile([128, CC, S], bf16)        # Q^T: head h at [(h%2)*64, h//2]
        kt_sb = big.tile([128, CC, S], bf16)        # K^T: same layout
        va_sb = big.tile([128, SKC, HG, DH + 1], bf16)  # [v*em, em] per key/head
        cx_sb = big.tile([128, CC, S], bf16)        # normalized ctx^T

        # ones column of va = exp(-1e9*mask) per key
        nc.sync.dma_start(va_sb[:, :, :, DH], emask8[:])

        # ================= phase A: projections =================
        def project_T(w_dram, brow, x_dram, dst_sb):
            """Q^T / K^T [512, 2048] = w_g^T @ x^T, bias via rank-1 matmul."""
            wt = wpool.tile([128, KC, DG], bf16, tag="w", name="wt")
            nc.sync.dma_start(wt, w_dram[:].rearrange("(ko p) c -> p ko c", p=128))
            for blk in range(4):
                xT = xtp.tile([128, KC, 512], bf16, tag="xT", name="xT")
                nc.sync.dma_start(xT, x_dram[:, blk * 512:(blk + 1) * 512]
                                  .rearrange("(ko p) s -> p ko s", p=128))
                for cc in range(CC):
                    ps = psx.tile([128, 512], f32, tag="psx", name="ps")
                    for kc in range(KC):
                        nc.tensor.matmul(ps, lhsT=wt[:, kc, cc * 128:(cc + 1) * 128],
                                         rhs=xT[:, kc, :],
                                         start=(kc == 0), stop=False)
                    nc.tensor.matmul(ps, lhsT=b2_sb[brow:brow + 1, cc * 128:(cc + 1) * 128],
                                     rhs=ones[brow:brow + 1, 0:512],
                                     start=False, stop=True)
                    with nc.allow_low_precision(reason="proj rounded to bf16"):
                        nc.vector.tensor_copy(
                            dst_sb[:, cc, blk * 512:(blk + 1) * 512], ps)

        project_T(wq, 0, xq, qt_sb)
        project_T(wk, 32, xk, kt_sb)

        # V: [keys, 8h*64] scaled by emask per key, interleaved into va
        wvt = wpool.tile([128, KC, DG], bf16, tag="w", name="wvt")
        nc.sync.dma_start(wvt, wv[:].rearrange("(ko p) c -> p ko c", p=128))
        for sc in range(SKC):
            xvt = xtp.tile([128, KC, 128], bf16, tag="xT", name="xvt")
            nc.sync.dma_start(xvt, xv[:, sc * 128:(sc + 1) * 128]
                              .rearrange("(ko p) s -> p ko s", p=128))
            ps = psx.tile([128, 512], f32, tag="psx", name="ps")
            for kc in range(KC):
                nc.tensor.matmul(ps, lhsT=xvt[:, kc, :], rhs=wvt[:, kc, :],
                                 start=(kc == 0), stop=(kc == KC - 1))
            with nc.allow_low_precision(reason="va in bf16"):
                nc.vector.tensor_scalar_mul(
                    va_sb[:, sc, :, 0:DH],
                    ps.rearrange("p (h d) -> p h d", h=HG),
                    em_sb[:, sc:sc + 1])

        # ================= phase B: pipelined attention =================
        state = {}

        def emit_logits_pair(t, kcp):
            st_ = state[t]
            h, sqb = st_["h"], st_["sqb"]
            hp, hcc = (h % 2) * 64, h // 2
            ps_ = psl.tile([128, 1024], f32, tag="psl", name="psl")
            for half in range(2):
                skc = kcp * 2 + half
                nc.tensor.matmul(ps_[:, half * 512:(half + 1) * 512],
                                 lhsT=kt_sb[hp:hp + 64, hcc,
                                            skc * 128:(skc + 1) * 128],
                                 rhs=qt_sb[hp:hp + 64, hcc,
                                           sqb * 512:(sqb + 1) * 512],
                                 start=True, stop=True)
            pt = ptp.tile([128, 2, 512], bf16, tag="pt", name="pt")
            nc.scalar.activation(pt.rearrange("p a b -> p (a b)"), ps_, Exp,
                                 scale=0.125)
            st_["pt"].append(pt)

        def emit_ctx_chunk(t, skc):
            st_ = state[t]
            if skc == 0:
                st_["psc"] = psc.tile([128, 512], f32, tag="psc", name="psc")
            nc.tensor.matmul(st_["psc"][0:DH + 1, :],
                             lhsT=va_sb[:, skc, st_["h"], :],
                             rhs=st_["pt"][skc // 2][:, skc % 2, :],
                             start=(skc == 0), stop=(skc == SKC - 1))

        def emit_norm(t):
            st_ = state[t]
            h, sqb = st_["h"], st_["sqb"]
            hcc, odd = h // 2, h % 2
            cu = stg.tile([DH + 1, 512], f32, tag="cu", name="cu")
            nc.vector.tensor_copy(cu, st_["psc"][0:DH + 1, :])
            rec = rcp.tile([1, 512], bf16, tag="rec", name="rec")
            with nc.allow_low_precision(reason="recip rounded to bf16"):
                nc.vector.reciprocal(rec, cu[DH:DH + 1, :])
            bc = psx.tile([128, 512], f32, tag="psx", name="bc")
            nc.tensor.matmul(bc[0:64, :], lhsT=ones[0:1, 0:64], rhs=rec[:],
                             start=True, stop=True)
            dst = cx_sb[64 * odd:64 * odd + 64, hcc, sqb * 512:(sqb + 1) * 512]
            with nc.allow_low_precision(reason="ctxn in bf16"):
                if not odd:
                    nc.vector.tensor_mul(out=dst, in0=cu[0:DH, :],
                                         in1=bc[0:64, :])
                else:
                    tm = tmp.tile([64, 512], bf16, tag="tmp", name="tm")
                    nc.vector.tensor_mul(out=tm, in0=cu[0:DH, :],
                                         in1=bc[0:64, :])
                    sh = psx.tile([128, 512], f32, tag="psx", name="sh")
                    nc.tensor.matmul(sh, lhsT=id_sb[:], rhs=tm[:],
                                     start=True, stop=True)
                    nc.vector.tensor_copy(dst, sh[64:128, :])
            del state[t]

        for t in range(NT):
            h, sqb = divmod(t, 4)
            state[t] = {"h": h, "sqb": sqb, "pt": []}
            for kcp in range(SKC // 2):
                emit_logits_pair(t, kcp)
                if t >= 1:
                    emit_ctx_chunk(t - 1, kcp * 2)
                    emit_ctx_chunk(t - 1, kcp * 2 + 1)
            # norm lags the ctx accumulation by a full iteration so the PE
            # never waits on the DVE reciprocal chain (HAM would re-throttle)
            if t >= 2:
                emit_norm(t - 2)
        for skc in range(SKC):
            emit_ctx_chunk(NT - 1, skc)
        emit_norm(NT - 2)
        emit_norm(NT - 1)

        # ================= phase C: output projection =================
        wot = wpool.tile([128, CC, D], bf16, tag="w", name="wot")
        nc.sync.dma_start(wot, wo[:].rearrange("(co p) c -> p co c", p=128))
        for st8 in range(SKC):
            ot = stg.tile([128, 1024], f32, tag="ost", name="ot")
            for half in range(2):
                ps = psx.tile([128, 512], f32, tag="psx", name="ps")
                for cc in range(CC):
                    nc.tensor.matmul(ps,
                                     lhsT=cx_sb[:, cc, st8 * 128:(st8 + 1) * 128],
                                     rhs=wot[:, cc, half * 512:(half + 1) * 512],
                                     start=(cc == 0), stop=(cc == CC - 1))
                nc.vector.tensor_copy(ot[:, half * 512:(half + 1) * 512], ps)
            nc.sync.dma_start(out[st8 * 128:(st8 + 1) * 128, :], ot)


_NC_CACHE = None


def kernel(query, key, value, mask, wq, bq, wk, bk, wv, bv, wo, bo):
    global _NC_CACHE
    if _NC_CACHE is None:
        _NC_CACHE = _build()
    nc = _NC_CACHE

    query = np.asarray(query, dtype=np.float32)
    key = np.asarray(key, dtype=np.float32)
    value = np.asarray(value, dtype=np.float32)
    mask = np.asarray(mask, dtype=np.float32)
    wq_np = np.asarray(wq, np.float32)
    wk_np = np.asarray(wk, np.float32)
    wv_np = np.asarray(wv, np.float32)
    wo_np = np.asarray(wo, np.float32)
    bq_np = np.asarray(bq, np.float32)
    bk_np = np.asarray(bk, np.float32)
    # fold bv and bo through the output projection (added on host at the end)
    bias_out = (np.asarray(bo, np.float64) +
                np.asarray(bv, np.float64) @ np.asarray(wo_np, np.float64)
                ).astype(np.float32)

    xT = {}
    for b in range(B):
        xT[b] = (np.ascontiguousarray(query[b].T).astype(np_bf16),
                 np.ascontiguousarray(key[b].T).astype(np_bf16),
                 np.ascontiguousarray(value[b].T).astype(np_bf16))
    shared_g = []
    for g in range(2):
        cols = slice(DG * g, DG * (g + 1))
        b2_host = np.zeros((33, DG), np.float32)
        b2_host[0] = bq_np[cols]
        b2_host[32] = bk_np[cols]
        shared_g.append({
            "wq": np.ascontiguousarray(wq_np[:, cols]).astype(np_bf16),
            "wk": np.ascontiguousarray(wk_np[:, cols]).astype(np_bf16),
            "wv": np.ascontiguousarray(wv_np[:, cols]).astype(np_bf16),
            "wo": np.ascontiguousarray(wo_np[cols, :]).astype(np_bf16),
            "b2": b2_host.astype(np_bf16),
        })
    one_host = np.ones((33, 512), np_bf16)
    id_host = np.concatenate([np.zeros((64, 64), np.float32),
                              np.eye(64, dtype=np.float32)],
                             axis=1).astype(np_bf16)

    in_maps = []
    for core in range(N_CORES):
        b, g = divmod(core, 2)
        em = np.exp(mask[b, 0, 0] * np.float32(-1e9)).astype(np.float32)
        emc = np.ascontiguousarray(em.reshape(SKC, 128).T)   # [128, SKC]
        em8 = np.ascontiguousarray(
            np.repeat(emc[:, :, None], HG, axis=2)).astype(np_bf16)
        in_maps.append({
            "xq": xT[b][0], "xk": xT[b][1], "xv": xT[b][2],
            "emask8": em8, "emaskf": emc,
            "one": one_host, "ident": id_host,
            **shared_g[g],
        })

    res = run_bass_kernel_spmd(nc, in_maps, core_ids=list(range(N_CORES)))
    full = np.empty((B, S, D), np.float32)
    for b in range(B):
        full[b] = res.results[2 * b]["out"]
        full[b] += res.results[2 * b + 1]["out"]
        full[b] += bias_out
    return full
